# revision 53
# baseline (speedup 1.0000x reference)
"""Trainium2 Bass kernel for nn_Activation1d (upsample2x-linear -> SiLU -> downsample2x).

Math: with align_corners=False linear resize, UP_RATIO=2, the whole op reduces
to a 3-tap stencil along T:
    a[j] = 0.75*x[j] + 0.25*x[j-1]      (x[-1] clamped to x[0])
    b[j] = 0.75*x[j] + 0.25*x[j+1]      (x[T] clamped to x[T-1])
    out[j] = 0.5*(silu(a[j]) + silu(b[j]))

Pure pointwise over (B, C): shard B*C = 8192 rows across 8 cores, T stays local.

Engine assignment (per [128, W] chunk), all DVE ops in fast perf modes:
    q  = 0.25*x[j-1..j+W]   tensor_scalar f32->f16, covers BOTH stencil taps:
                            q[i] = 0.25*x[lo-1+i], so the j-1 tap is q[0:W]
                            (offset 0) and the j+1 tap is q[2:W+2] (offset 2,
                            still 4B-aligned in f16 -- the whole point of the
                            shifted-by-one layout; a +-1 f16 shift would break
                            the 2x packed mode).
    m  = 0.75*x[j]          tensor_scalar f32->f16 (f32 reads are 4B-aligned
                            at any element offset, so shifted slices of the
                            raw f32 row are free).
    a, b = m + q-taps       tensor_tensor f16 (2x mode)
    silu(a), silu(b)        ACT engine, in place
    s  = a + b              tensor_tensor f16 (2x mode), in place
    out = 0.5*s -> f32      tensor_scalar (DVE) or Copy-activation (ACT),
                            K_HALF balances the two engines.

The old kernel did this with scalar_tensor_tensor (no fast DVE uops: 5.2us per
chunk vs 1.1us for tensor_tensor) plus a full Pool-engine shifted copy (33us
per row-tile); DVE was 90% busy and the kernel ran 2.7x over the DMA roofline.
"""

import os
import sys
from contextlib import ExitStack

import numpy as np

for _p in ("/opt/trn_rl_repo",):
    if _p not in sys.path:
        sys.path.insert(0, _p)

import bass_rust
import concourse.bass as bass
import concourse.mybir as mybir
from concourse import tile
from concourse.bass_utils import run_bass_kernel_spmd

N_CORES = 8
B, C, T = 16, 512, 8192
ROWS = B * C                 # 8192
RPC = ROWS // N_CORES        # 1024 rows per core
P = 128                      # SBUF partitions
N_RT = RPC // P              # 8 row-tiles per core

ALU = mybir.AluOpType
AFT = mybir.ActivationFunctionType
F32 = mybir.dt.float32

# --- tunables (env-overridable for experiments) ---
CDT_NAME = os.environ.get("K_CDT", "float16")       # compute dtype for DVE ops
W = int(os.environ.get("K_W", "2048"))               # free-dim compute chunk width
# Engine load-balance knobs. "half" (the final 0.5*(sa+sb) -> f32 op) must
# stay on DVE: an ACT writer of oc would carry two non-transitive waits
# (DVE s-add + store-DMA buffer reuse) and trip the single-wait ISA limit.
# "m" (0.75*x f32->f16) can go to ACT: its waits (load DMA + m-buf reuse)
# are implied by earlier DVE events, so the prune pass collapses them.
# With the PE offload (K_PE=1) the balance flips: ACT keeps the silus plus
# most PSUM evacs, so m belongs on DVE.
HALF_ENG = os.environ.get("K_HALF", "dve")
M_ENG = os.environ.get("K_M", "dve" if os.environ.get("K_PE", "0") == "1" else "act")
# Inputs via SWDGE (gpsimd); outputs MUST be HWDGE (sync): compute
# instructions waiting on an SWDGE out-DMA's lane semaphore hang the device
# (the +16 never lands), while the same WAR pattern on DMAHW lanes works.
OUT_DMA_ENGINE = os.environ.get("K_ODMA", "sync")
IN_DMA_ENGINE = os.environ.get("K_IDMA", "gpsimd")
# Probe: split rt0's load / rt7's store in two (9 DMAs per DGE type, lane 0
# reused) to shrink pipeline ramp/tail. CONFIRMED BROKEN on this stack (the
# 9th DMA's completion never lands and the NEFF hangs) -- keep 0.
SPLIT_EDGE = os.environ.get("K_SPLIT", "0") == "1"
# Fuse the a/b adds into one 2W-wide tensor_tensor (stride-0 broadcast of m,
# stride-2 double-tap view of q) and the two silus into one 2W ACTIVATE.
# Saves one 58-cycle DVE init + one 352-cycle ACT init per chunk, but the
# coarser ops pipeline worse across DVE->ACT->DVE (measured ~5us slower
# end-to-end than separate ops despite lower ACT busy) -- default off.
FUSE_AB = os.environ.get("K_FUSE", "0") == "1"
# Offload the final 0.5*(sa+sb) to the (otherwise idle) PE engine as two
# accumulating matmuls against a constant 0.5*I weight; the PSUM result is
# evacuated by ACT (most row-tiles) or DVE (PE_DVE_RTS row-tiles), balancing
# both engines ~20us below the previous 187us wall. DISABLED: compiles and
# passes CoreSim + the wait pruner, but the NEFF hangs on this hardware
# stack (suspect PSUM accumulation-group or PE event-accel semantics).
USE_PE = os.environ.get("K_PE", "0") == "1"
MMQ = 512                       # matmul N per instruction (one PSUM bank)
# Row-tiles whose PSUM evac runs on DVE (engine balance). rt7 is one of them
# so the donor copies (reading rt7's oc, same-engine dep) schedule after all
# real DVE work.
PE_DVE_RTS = {2, 7}

NCH = T // W                 # chunks per row-tile

_LAST_EXEC_NS = None
_LAST_RESULT = None


def _build():
    cdt = getattr(mybir.dt, CDT_NAME)
    # Tile's stale SBUF cap (192K) leaves real capacity (208K usable) unused;
    # the W=4096 configuration needs ~194K per partition.
    import concourse.tile_utils as _tu

    _tu.max_sbuf_usage = 208 * 1024
    nc = bass.Bass()
    x_ext = nc.declare_dram_parameter("x", [RPC, T], F32, isOutput=False)
    o_ext = nc.declare_dram_parameter("out", [RPC, T], F32, isOutput=True)

    with tile.TileContext(nc) as tc:
        with ExitStack() as ctx:
            xpool = ctx.enter_context(tc.tile_pool(name="xp", bufs=2))
            qpool = ctx.enter_context(tc.tile_pool(name="qp", bufs=2))
            mpool = ctx.enter_context(tc.tile_pool(name="mp", bufs=2))
            abpool = ctx.enter_context(
                tc.tile_pool(name="ab", bufs=4 if W <= 2048 else 2)
            )
            opool = ctx.enter_context(tc.tile_pool(name="op", bufs=2))

            in_dma = getattr(nc, IN_DMA_ENGINE)
            out_dma = getattr(nc, OUT_DMA_ENGINE)
            ts = nc.vector.tensor_scalar
            tt = nc.vector.tensor_tensor

            heye = None
            ppool = None
            if USE_PE:
                epool = ctx.enter_context(tc.tile_pool(name="eye", bufs=1))
                heye = epool.tile([P, P], cdt, tag="eye")
                # 0.5*I built once on Pool: memset 0 then fill the diagonal.
                nc.gpsimd.memset(heye[:], 0.0)
                nc.gpsimd.affine_select(
                    out=heye[:],
                    in_=heye[:],
                    compare_op=ALU.not_equal,
                    fill=0.5,
                    base=0,
                    pattern=[[-1, P]],
                    channel_multiplier=1,
                )
                ppool = ctx.enter_context(
                    tc.tile_pool(name="ps", bufs=2, space=bass.MemorySpace.PSUM)
                )

            # DMA budget: broken DGE lane-reuse in this stack means at most 8
            # DMAs per ring (SWDGE qPoolDynamic / HWDGE qSPDynamicHW) so no
            # lane is ever reused: 8 full-row loads (SWDGE) + 8 full-row
            # stores (sync HWDGE).
            #
            # The chunk loop is software-pipelined: chunk ci's tail (s-add +
            # half) is issued AFTER chunk ci+1's head (q/m/a/b + silu), so
            # the in-order DVE stream never stalls waiting for ACT's silu --
            # by the time DVE reaches s(ci) the silu has had a full chunk of
            # ACT time to finish.
            ocs = {}

            stores = {}

            def head(r, ci):
                xt, xb, oc, ob = ocs[r][ci]
                lo = ci * W
                # q[i] = 0.25*x[lo-1+i] for i in [0, W+2), edges clamped.
                # The f32 source tolerates any element offset; the f16
                # destination is written at even offsets by the wide ops
                # (DVE 16-bit stores need 4B alignment for the packed 2x
                # mode). The two lone edge elements are 1-wide DVE writes
                # on the same engine, so they add no cross-engine waits.
                q = qpool.tile([P, W + 2], cdt, tag="q")
                if ci == 0:
                    ts(q[:, 2 : W + 2], xt[:, 1 - xb : W + 1 - xb], 0.25, None, ALU.mult)
                    ts(q[:, 0:1], xt[:, 0 - xb : 1 - xb], 0.25, None, ALU.mult)
                    ts(q[:, 1:2], xt[:, 0 - xb : 1 - xb], 0.25, None, ALU.mult)
                elif ci == NCH - 1:
                    ts(q[:, 0:W], xt[:, lo - 1 - xb : lo + W - 1 - xb], 0.25, None, ALU.mult)
                    ts(q[:, W : W + 1], xt[:, T - 1 - xb : T - xb], 0.25, None, ALU.mult)
                    ts(q[:, W + 1 : W + 2], xt[:, T - 1 - xb : T - xb], 0.25, None, ALU.mult)
                else:
                    ts(q[:, 0 : W + 2], xt[:, lo - 1 - xb : lo + W + 1 - xb], 0.25, None, ALU.mult)

                m = mpool.tile([P, W], cdt, tag="m")
                m_act = M_ENG == "act" or (M_ENG == "alt" and ci % 2 == 0)
                if m_act:
                    nc.scalar.activation(m[:], xt[:, lo - xb : lo + W - xb], AFT.Copy, scale=0.75)
                else:
                    ts(m[:], xt[:, lo - xb : lo + W - xb], 0.75, None, ALU.mult)

                if FUSE_AB:
                    ab = abpool.tile([P, 2 * W], cdt, tag="ab")
                    ab_v = ab[:].rearrange("p (two w) -> p two w", two=2)
                    m_v = m[:].unsqueeze(1).broadcast_to([P, 2, W])
                    q_v = q[:]
                    q_v.ap = mybir.VecI64Pair([[W + 2, P], [2, 2], [1, W]])
                    tt(ab_v, m_v, q_v, ALU.add)
                    # silu in place (1:1 elementwise, no RAW hazard)
                    nc.scalar.activation(ab[:], ab[:], AFT.Silu)
                    if USE_PE:
                        # psum[:, t] = 0.5*sa[:, t] + 0.5*sb[:, t], one PSUM
                        # bank (512 f32) per accumulation group.
                        ps = ppool.tile([P, W], F32, tag="ps", space=bass.MemorySpace.PSUM)
                        for t in range(W // MMQ):
                            sl = slice(t * MMQ, (t + 1) * MMQ)
                            sl_b = slice(W + t * MMQ, W + (t + 1) * MMQ)
                            nc.tensor.matmul(
                                ps[:, sl], heye[:], ab[:, sl],
                                start=True, stop=False,
                            )
                            nc.tensor.matmul(
                                ps[:, sl], heye[:], ab[:, sl_b],
                                start=False, stop=True,
                            )
                        return ps
                    return (ab[:, 0:W], ab[:, W : 2 * W])
                a = abpool.tile([P, W], cdt, tag="ab")
                b = abpool.tile([P, W], cdt, tag="ab")
                tt(a[:], m[:], q[:, 0:W], ALU.add)
                tt(b[:], m[:], q[:, 2 : W + 2], ALU.add)
                # silu in place (1:1 elementwise, no RAW hazard)
                nc.scalar.activation(a[:], a[:], AFT.Silu)
                nc.scalar.activation(b[:], b[:], AFT.Silu)
                return (a[:], b[:])

            def tail(r, ci, ab):
                xt, xb, oc, ob = ocs[r][ci]
                lo = ci * W
                if USE_PE:
                    ps = ab
                    # PSUM -> SBUF evacuation; engine fixed per row-tile so
                    # consecutive oc writers stay in one stream (single wait).
                    if r in PE_DVE_RTS:
                        nc.vector.tensor_copy(oc[:, lo - ob : lo + W - ob], ps[:])
                    else:
                        nc.scalar.activation(
                            oc[:, lo - ob : lo + W - ob], ps[:], AFT.Copy
                        )
                else:
                    a, b = ab
                    tt(a, a, b, ALU.add)
                    use_act = HALF_ENG == "act" or (HALF_ENG == "alt" and ci % 2 == 0)
                    if use_act:
                        nc.scalar.activation(
                            oc[:, lo - ob : lo + W - ob], a, AFT.Copy, scale=0.5
                        )
                    else:
                        ts(oc[:, lo - ob : lo + W - ob], a, 0.5, None, ALU.mult)
                for dram_lo, dram_hi, st_oc, st_ob in stores.get((r, ci), ()):
                    rows = slice(r * P, (r + 1) * P)
                    out_dma.dma_start(
                        o_ext[rows, dram_lo:dram_hi],
                        st_oc[:, dram_lo - st_ob : dram_hi - st_ob],
                    )

            pending = None
            H = T // 2
            for r in range(N_RT):
                rows = slice(r * P, (r + 1) * P)
                if SPLIT_EDGE and r == 0:
                    # Two half loads into separate tiles (overlapping by 2
                    # columns for the stencil halo) so chunk 0's compute can
                    # start after ~half the load latency. The 9th SWDGE load
                    # then reuses lane 0, exercising the DGE lane-reuse path.
                    xa = xpool.tile([P, H + 2], F32, tag="xt")
                    in_dma.dma_start(xa[:], x_ext[rows, 0 : H + 2])
                    xc = xpool.tile([P, H + 2], F32, tag="xt")
                    in_dma.dma_start(xc[:], x_ext[rows, H - 2 : T])
                    oc = opool.tile([P, T], F32, tag="oc")
                    percol = {}
                    for ci in range(NCH):
                        src = (xa, 0) if (ci + 1) * W <= H else (xc, H - 2)
                        percol[ci] = (src[0], src[1], oc, 0)
                    ocs[r] = percol
                    stores[(r, NCH - 1)] = [(0, T, oc, 0)]
                elif SPLIT_EDGE and r == N_RT - 1:
                    xt = xpool.tile([P, T], F32, tag="xt")
                    in_dma.dma_start(xt[:], x_ext[rows, :])
                    # Two half-width output tiles; the first half's store
                    # fires as soon as its chunks are done, pulling the
                    # final store off the critical tail.
                    oa = opool.tile([P, H], F32, tag="oc")
                    ob_t = opool.tile([P, H], F32, tag="oc")
                    percol = {}
                    for ci in range(NCH):
                        dst = (oa, 0) if (ci + 1) * W <= H else (ob_t, H)
                        percol[ci] = (xt, 0, dst[0], dst[1])
                    ocs[r] = percol
                    mid_ci = (H // W) - 1
                    stores[(r, mid_ci)] = [(0, H, oa, 0)]
                    stores[(r, NCH - 1)] = [(H, T, ob_t, H)]
                else:
                    xt = xpool.tile([P, T], F32, tag="xt")
                    in_dma.dma_start(xt[:], x_ext[rows, :])
                    oc = opool.tile([P, T], F32, tag="oc")
                    ocs[r] = {ci: (xt, 0, oc, 0) for ci in range(NCH)}
                    stores[(r, NCH - 1)] = [(0, T, oc, 0)]
                if USE_PE:
                    # Seed write: a 1-wide DVE touch of oc absorbs the
                    # store-DMA buffer-reuse wait into the DVE stream, so the
                    # first PSUM evac of the row-tile strengthens to a single
                    # PE wait (PE knows the DVE stream through silu -> ab).
                    nc.vector.memset(ocs[r][0][2][:, 0:1], 0.0)
                for ci in range(NCH):
                    ab = head(r, ci)
                    if pending is not None:
                        tail(*pending)
                    pending = (r, ci, ab)
            tail(*pending)
            if USE_PE:
                last_oc = ocs[7][0][2]
            # Donor fodder for the prune pass: zero-wait tail instructions
            # that phase 3 can re-point at surplus semaphore waits. Plain
            # zero-dep memsets get scheduled EARLY by Tile (unsound as
            # donors); these copies read a DVE-evacuated oc tile, giving
            # them a same-engine RAW dep (no semaphore) that pins them after
            # the end of real DVE work.
            if USE_PE:
                spool = ctx.enter_context(tc.tile_pool(name="sp", bufs=1))
                scratch = spool.tile([P, 64], F32, tag="scr")
                for i in range(32):
                    nc.vector.tensor_copy(
                        scratch[:, i : i + 1], last_oc[:, i : i + 1]
                    )
    return nc


_PRUNABLE = tuple(
    t
    for t in (
        bass_rust.InstDMACopy,
        bass_rust.InstTensorCopy,
        bass_rust.InstTensorTensor,
        bass_rust.InstTensorScalarPtr,
        bass_rust.InstActivation,
        getattr(bass_rust, "InstMatmult", None),
        getattr(bass_rust, "InstMemset", None),
    )
    if t is not None
)


def _transitive_prune_waits(nc):
    """Reduce every prunable instruction to at most one semaphore wait.

    This walrus build's engine/DMA ISA structs hold a single sync wait per
    instruction, but Tile's scheduler emits one wait per dependent proc
    because its vector clock is not transitively minimal across procs.

    Phase 1 simulates the emitted program (greedy topological execution over
    per-engine in-order streams), recording for every semaphore value the
    happens-before knowledge it implies and a global feasible order.
    Phase 2 drops waits implied by program order + remaining waits; if more
    than one wait survives, it strengthens one wait (raising its threshold
    to a value already reached earlier in the phase-1 order, so no cycle can
    form) until that single wait implies all the others.

    Soundness: engines complete instructions in stream order (DVE/ACT/SP);
    per-lane DMA updates land in issue order (Tile serializes lane reuse);
    Pool compute may complete out of order across Q7 cores, so no transitive
    knowledge is propagated through the Pool semaphore.
    """
    f = nc.m.functions[0]
    streams = {}
    for b in f.blocks:
        for inst in b.instructions:
            streams.setdefault(str(inst.engine), []).append(inst)

    def merge(dst, src):
        for s, v in src.items():
            if dst.get(s, 0) < v:
                dst[s] = v

    # ---- phase 1: simulate, collect logs ----
    sem_val = {}
    sem_log = {}        # sem -> list of (cum_value, knowledge, step)
    proc_know = {e: {} for e in streams}
    proc_self = {e: {} for e in streams}
    ptr = {e: 0 for e in streams}
    inst_info = {}      # id(inst) -> (base knowledge, step)
    step = 0

    def knowledge_of(sem, val, max_step=None):
        k = {sem: val}
        # Pool (8 Q7 cores) and PE (matmuls pipelined across PSUM banks)
        # complete out of order: a semaphore value on them implies nothing
        # about which specific instructions finished.
        if sem.startswith("Pool") or sem.startswith("PE"):
            return k
        for cum, kn, st in sem_log.get(sem, ()):
            if max_step is not None and st >= max_step:
                break
            merge(k, kn)
            if cum >= val:
                break
        return k

    def satisfied(w):
        v = sem_val.get(w.ant_name, 0)
        return v == w.wait_value if w.wait_mode == "sem-eq-imm" else v >= w.wait_value

    def execute(eng, inst):
        nonlocal step, done
        si = inst.sync_info
        waits = list(si.on_wait) if si is not None else []
        base = dict(proc_know[eng])
        merge(base, proc_self[eng])
        inst_info[id(inst)] = (dict(base), step)
        acc = base
        for w in waits:
            merge(acc, knowledge_of(w.ant_name, w.wait_value))
        proc_know[eng] = acc
        is_dma = isinstance(inst, bass_rust.InstDMACopy)
        if si is not None:
            for u in si.on_update:
                s = u.ant_name
                dv = {
                    "sem-add-imm": u.update_value,
                    "sem-inc": 1,
                    "sem-dec": -1,
                    "sem-sub-imm": -u.update_value,
                }[u.update_mode]
                nv = sem_val.get(s, 0) + dv
                sem_val[s] = nv
                kn = dict(proc_know[eng])
                merge(kn, proc_self[eng])
                if not is_dma and eng not in ("EngineType.Pool", "EngineType.PE"):
                    # Pool (8 Q7 cores) and PE (PSUM-bank ILP) complete out
                    # of order: a later instruction on them cannot assume
                    # earlier ones finished.
                    proc_self[eng][s] = max(proc_self[eng].get(s, 0), nv)
                kn[s] = nv
                sem_log.setdefault(s, []).append((nv, kn, step))
        ptr[eng] += 1
        done += 1
        step += 1

    total = sum(len(s) for s in streams.values())
    done, progress = 0, True
    while done < total and progress:
        progress = False
        # Execute DMAs as late as possible so compute events order before
        # them in the recorded feasible order (maximizes strengthening).
        for eng, stream in streams.items():
            while ptr[eng] < len(stream):
                inst = stream[ptr[eng]]
                si = inst.sync_info
                waits = list(si.on_wait) if si is not None else []
                if isinstance(inst, bass_rust.InstDMACopy):
                    break
                if not all(satisfied(w) for w in waits):
                    break
                execute(eng, inst)
                progress = True
        if progress:
            continue
        # Prefer store (SP/HWDGE) DMAs over load (Pool/SWDGE) DMAs when
        # stuck: stores unblock downstream compute (oc buffer reuse), which
        # pushes the loads' sim steps later and lets phase 2 find a single
        # compute-sem event that transitively implies all of a load's waits.
        for eng in sorted(streams, key=lambda e: e == "EngineType.Pool"):
            stream = streams[eng]
            if ptr[eng] < len(stream):
                inst = stream[ptr[eng]]
                si = inst.sync_info
                waits = list(si.on_wait) if si is not None else []
                if isinstance(inst, bass_rust.InstDMACopy) and all(
                    satisfied(w) for w in waits
                ):
                    execute(eng, inst)
                    progress = True
                    break
    if done < total:
        import logging

        logging.warning(
            "_transitive_prune_waits: simulation stalled at %d/%d; "
            "no pruning applied",
            done,
            total,
        )
        return

    # ---- phase 2: prune / strengthen ----
    remaining_multi = []
    for eng, stream in streams.items():
        for inst in stream:
            si = inst.sync_info
            waits = list(si.on_wait) if si is not None else []
            if len(waits) < 2:
                continue
            if not isinstance(inst, _PRUNABLE) or any(
                w.wait_mode != "sem-ge-imm" for w in waits
            ):
                remaining_multi.append(inst)
                continue
            base, my_step = inst_info[id(inst)]

            def implied(k, ws):
                return all(k.get(w.ant_name, 0) >= w.wait_value for w in ws)

            # A DMA's wait on its own update lane (Tile's lane-reuse
            # throttle) is load-bearing for the DGE hardware beyond its
            # ordering semantics: dropping it wedges the device even when
            # the ordering is transitively guaranteed. Never touch those.
            own_lanes = set()
            if isinstance(inst, bass_rust.InstDMACopy) and si is not None:
                own_lanes = {u.ant_name for u in si.on_update}
            fixed = [w for w in waits if w.ant_name in own_lanes]
            # 1) drop waits implied by base + the other waits (greedy, all orders)
            import itertools

            best = None
            for order in itertools.permutations(range(len(waits))):
                a = dict(base)
                for w in fixed:
                    merge(a, knowledge_of(w.ant_name, w.wait_value))
                kp = [i for i in range(len(waits)) if waits[i] in fixed]
                for i in order:
                    w = waits[i]
                    if w in fixed:
                        continue
                    if a.get(w.ant_name, 0) >= w.wait_value:
                        continue
                    kp.append(i)
                    merge(a, knowledge_of(w.ant_name, w.wait_value))
                if best is None or len(kp) < len(best):
                    best = kp
                if len(kp) <= 1:
                    break
            kept = [waits[i] for i in sorted(best)]
            # 2) strengthen: find one sem whose (possibly later) value implies all
            if len(kept) > 1 and fixed:
                remaining_multi.append(inst)
                continue
            if len(kept) > 1:
                chosen = None
                cands = sorted(
                    {w.ant_name for w in waits},
                    key=lambda s: (s.startswith("DMA"), s),
                )
                for s in cands:
                    if s.startswith("Pool") or s.startswith("PE"):
                        continue
                    k = dict(base)
                    for cum, kn, st in sem_log.get(s, ()):
                        if st >= my_step:
                            break  # only events already ordered before us
                        merge(k, kn)
                        k[s] = max(k.get(s, 0), cum)
                        if implied(k, waits):
                            chosen = (s, cum)
                            break
                    if chosen:
                        break
                if chosen:
                    tmpl = next(w for w in waits if w.ant_name == chosen[0])
                    tmpl.wait_value = chosen[1]
                    kept = [tmpl]
                else:
                    remaining_multi.append(inst)
                    continue
            if len(kept) != len(waits) or any(
                k.wait_value != w.wait_value for k, w in zip(kept, waits)
            ):
                si.on_wait = kept
                inst.sync_info = si
    # ---- phase 3: non-prunable multi-wait instructions (the tail drain) ----
    # Reduce to the minimal wait subset via transitivity, keep one wait, and
    # move the rest onto zero-wait tail instructions (event semaphores) that
    # execute before NEFF completion. Sound: the conditions depend only on
    # DMAs issued in the main region, so no donor can deadlock, and every
    # stream must finish before the NEFF signals done.
    import itertools as _it

    unresolved = []
    if remaining_multi:
        last_dma_step = max(
            (inst_info[id(i)][1] for s in streams.values() for i in s
             if isinstance(i, bass_rust.InstDMACopy) and id(i) in inst_info),
            default=0,
        )
        donors = [
            i
            for s in streams.values()
            for i in s
            if isinstance(
                i, (bass_rust.InstEventSemaphore, bass_rust.InstDrain)
            )
            and i.sync_info is not None
            and not list(i.sync_info.on_wait)
            and inst_info.get(id(i), (None, -1))[1] > last_dma_step
        ]
        # Zero-wait memsets/copies positioned after every other real
        # (data-producing) op of their stream are sound donors regardless of
        # sim step: nothing any semaphore producer depends on can come after
        # them, so parking a surplus wait there cannot form a cycle.
        sink_t = tuple(
            t
            for t in (
                getattr(bass_rust, "InstMemset", None),
                bass_rust.InstTensorCopy,
            )
            if t is not None
        )
        real_t = tuple(
            t
            for t in (
                bass_rust.InstDMACopy,
                bass_rust.InstTensorTensor,
                bass_rust.InstTensorScalarPtr,
                bass_rust.InstActivation,
                getattr(bass_rust, "InstMatmult", None),
            )
            if t is not None
        )
        for s in streams.values():
            last_real = max(
                (k for k, i in enumerate(s) if isinstance(i, real_t)),
                default=-1,
            )
            donors.extend(
                i
                for i in s[last_real + 1 :]
                if isinstance(i, sink_t)
                and i.sync_info is not None
                and not list(i.sync_info.on_wait)
            )
        # Small wait-sets first so the many-wait tail drain doesn't starve
        # the donor pool.
        remaining_multi.sort(key=lambda i: len(list(i.sync_info.on_wait)))
        for inst in remaining_multi:
            si = inst.sync_info
            waits = list(si.on_wait)
            if any(w.wait_mode != "sem-ge-imm" for w in waits):
                unresolved.append(inst)
                continue
            base, _st = inst_info[id(inst)]
            best = None
            for r in range(1, len(waits) + 1):
                for combo in _it.combinations(range(len(waits)), r):
                    k = dict(base)
                    for i in combo:
                        merge(k, knowledge_of(waits[i].ant_name, waits[i].wait_value))
                    if all(k.get(w.ant_name, 0) >= w.wait_value for w in waits):
                        best = [waits[i] for i in combo]
                        break
                if best:
                    break
            if best is None:
                best = waits
            extra = best[1:]
            if len(extra) > len(donors):
                unresolved.append(inst)
                continue
            for w in extra:
                d = donors.pop()
                dsi = d.sync_info
                dsi.on_wait = [w]
                d.sync_info = dsi
            si.on_wait = best[:1]
            inst.sync_info = si
    if unresolved:
        import logging

        logging.warning(
            "_transitive_prune_waits: %d instructions still multi-wait: %s",
            len(unresolved),
            [i.name for i in unresolved[:10]],
        )


_NC = None


def _get_nc():
    global _NC
    if _NC is None:
        _NC = _build()
        _transitive_prune_waits(_NC)
    return _NC


def kernel(x):
    global _LAST_EXEC_NS, _LAST_RESULT
    x = np.asarray(x, dtype=np.float32)
    assert x.shape == (B, C, T), x.shape
    flat = np.ascontiguousarray(x.reshape(ROWS, T))
    in_maps = [
        {"x": np.ascontiguousarray(flat[i * RPC : (i + 1) * RPC])}
        for i in range(N_CORES)
    ]
    nc = _get_nc()
    res = run_bass_kernel_spmd(
        nc,
        in_maps,
        core_ids=list(range(N_CORES)),
        trace=os.environ.get("K_TRACE", "0") == "1",
    )
    _LAST_RESULT = res
    _LAST_EXEC_NS = res.exec_time_ns
    out = np.concatenate([r["out"] for r in res.results], axis=0)
    return np.ascontiguousarray(out.reshape(B, C, T))
